# revision 6
# baseline (speedup 1.0000x reference)
"""Trainium2 Bass kernel for LowRankRayTracer.

csi[f] = (delta_t/D) * v_f^T M v_f,  M = conj(rad)^T conj(att)  (R=32, complex)
contracted over N = D*K = 524288 rows.

Strategy (8 cores, memory-bound => minimize HBM bytes):
  - Tolerance is 2e-2; fp16-only inputs give ~6e-4 (validated in sim), so
    each f32 component ships as ONE fp16 (half the hi+lo baseline's
    traffic): 16 MiB/core of ray data + ~2.3 MiB of frequency/const data.
    All input DMAs are issued up front into statically-resident SBUF
    tiles (20 MiB), so the input stream runs at the ~358 GB/s HBM limit
    without stalls.
  - Rows sharded 8 ways; host sums the 8 partial csi vectors (an
    on-device all-reduce was measured at ~60us on this path - dead end).
  - s128 = rad_pack^T att_pack accumulates in 4 PSUM banks for the first
    15.75 MiB; that partial is folded and pushed through the 16 +-1
    selection matmuls into W's PSUM accumulators while the last 0.5 MiB
    still streams, so only a small fold+16 matmuls remain on the post-DMA
    critical path (W = SEL(s_a) + SEL(s_b) - SEL is linear).
  - Phase 3 per 1024-col chunk: T = W^T g (PE), e = g (.) T (DVE, direct
    from PSUM), csi = ones^T e (PE, issue-interleaved with T so the
    in-order PE queue pipelines), (2,1024) csi copies on the otherwise
    idle ACT engine, output DMA in 4 overlapping pieces.
"""

import numpy as np

D, K, R = 4096, 128, 32
F = 8192
N_CORES = 8
DIR_PER_CORE = D // N_CORES              # 512
N_CHUNK = 4                              # macro chunks per tensor per core
CHUNK_COLS = 8192                        # fp16 per partition per chunk
SLICES = CHUNK_COLS // 128               # 64 matmul slices per chunk
SCALE = (200.0 / K) / D                  # delta_t / num_directions
FCHUNK = 1024
N_FCHUNK = F // FCHUNK                   # 8
SEL_COLS = 1152                          # 16*64 selection cols + ones2 + pad
LP1, LP2 = 6144, 7680                    # last-chunk DMA/PSUM split points

_NC_CACHE = {}


def _build_sel():
    """(128, 1152) f16: 16 (128,64) +-1 selection matrices + ones2 cols.

    W 32-block (R,C) = sum of sigma*Q_uv;  Q_uv = S64[32u:+32, 32v:+32],
    S64 = s128[0:64,0:64] + s128[64:,64:].  Matmul g uses rhs cols
    [32g:32g+32] of s128; source (side, v) -> g = v + 2*side.
    """
    table = {
        (0, 0): [(+1, 0, 0), (-1, 1, 1)],   # Mr
        (1, 0): [(+1, 0, 1), (+1, 1, 0)],   # -Mi
        (0, 1): [(+1, 0, 1), (+1, 1, 0)],   # -Mi
        (1, 1): [(-1, 0, 0), (+1, 1, 1)],   # -Mr
        (0, 2): [(-1, 0, 1), (-1, 1, 0)],   # Mi
        (1, 2): [(+1, 0, 0), (-1, 1, 1)],   # Mr
        (0, 3): [(+1, 0, 0), (-1, 1, 1)],   # Mr
        (1, 3): [(+1, 0, 1), (+1, 1, 0)],   # -Mi
    }
    sel = np.zeros((128, SEL_COLS), np.float16)
    for (Rb, C), terms in table.items():
        for sigma, u, v in terms:
            for side in (0, 1):
                g = v + 2 * side
                for r0 in range(32):
                    p = 64 * side + 32 * u + r0
                    sel[p, (C * 4 + g) * 64 + 32 * Rb + r0] = sigma
    sel[0:64, 1024] = 1.0
    sel[64:128, 1025] = 1.0
    return sel


def build_nc():
    import concourse.bacc as bacc
    import concourse.mybir as mybir
    import concourse.tile as tile

    fp32 = mybir.dt.float32
    fp16 = mybir.dt.float16
    nc = bacc.Bacc(trn_type="TRN2", target_bir_lowering=False, debug=False)

    rad_d = nc.dram_tensor("rad", [N_CHUNK, 128, CHUNK_COLS], fp16,
                           kind="ExternalInput").ap()
    att_d = nc.dram_tensor("att", [N_CHUNK, 128, CHUNK_COLS], fp16,
                           kind="ExternalInput").ap()
    gtd_d = nc.dram_tensor("gtd", [128, F], fp16, kind="ExternalInput").ap()
    sel_d = nc.dram_tensor("sel", [128, SEL_COLS], fp16,
                           kind="ExternalInput").ap()
    out_d = nc.dram_tensor("csi", [2, F], fp16, kind="ExternalOutput").ap()

    with tile.TileContext(nc) as tc:
        with (
            tc.tile_pool(name="io", bufs=1) as io_pool,
            tc.tile_pool(name="small", bufs=1) as small,
            tc.tile_pool(name="epool", bufs=1) as epool,
        ):
            sel_sb = small.tile([128, SEL_COLS], fp16, tag="sel")
            gtd_sb = small.tile([128, F], fp16, tag="gtd")

            rad_t = [io_pool.tile([128, CHUNK_COLS], fp16, tag=f"rad{t}",
                                  name=f"rad{t}")
                     for t in range(N_CHUNK)]
            att_t = [io_pool.tile([128, CHUNK_COLS], fp16, tag=f"att{t}",
                                  name=f"att{t}")
                     for t in range(N_CHUNK)]

            # ---- all input DMAs up front; rad on the sync HWDGE ring,
            # att on the scalar ring (rings drain round-robin, sharing the
            # 358 GB/s HBM limit). sel/gtd ride mid-stream so the epilogue
            # and phase 3 never wait on them; the last chunk is split so
            # only 0.125 MiB of matmul input lands after the stream ends.
            hm = CHUNK_COLS // 2
            nc.sync.dma_start(rad_t[0][:, 0:hm], rad_d[0, :, 0:hm])
            nc.scalar.dma_start(att_t[0][:, 0:hm], att_d[0, :, 0:hm])
            nc.sync.dma_start(rad_t[0][:, hm:], rad_d[0, :, hm:])
            nc.scalar.dma_start(att_t[0][:, hm:], att_d[0, :, hm:])
            nc.sync.dma_start(sel_sb[:], sel_d[:])
            nc.sync.dma_start(rad_t[1][:], rad_d[1, :, :])
            nc.scalar.dma_start(att_t[1][:], att_d[1, :, :])
            nc.sync.dma_start(gtd_sb[:, 0:4096], gtd_d[:, 0:4096])
            nc.scalar.dma_start(gtd_sb[:, 4096:8192], gtd_d[:, 4096:8192])
            nc.sync.dma_start(rad_t[2][:], rad_d[2, :, :])
            nc.scalar.dma_start(att_t[2][:], att_d[2, :, :])
            nc.sync.dma_start(rad_t[3][:, 0:LP1], rad_d[3, :, 0:LP1])
            nc.scalar.dma_start(att_t[3][:, 0:LP1], att_d[3, :, 0:LP1])
            nc.sync.dma_start(rad_t[3][:, LP1:LP2], rad_d[3, :, LP1:LP2])
            nc.scalar.dma_start(att_t[3][:, LP1:LP2], att_d[3, :, LP1:LP2])
            nc.sync.dma_start(rad_t[3][:, LP2:], rad_d[3, :, LP2:])
            nc.scalar.dma_start(att_t[3][:, LP2:], att_d[3, :, LP2:])

            # ---- main accumulation: s128 += rad_slice^T att_slice ----
            # chunks 0-2 + chunk3 cols [0:LP1] -> banks 0-3 ("a" part);
            # chunk3 cols [LP1:] -> 2 fresh tiles ("b" part) so the a-fold
            # and 16 a-selection matmuls overlap the tail DMAs.
            s16a = small.tile([128, 128], fp16, tag="s16a")
            s16b = small.tile([128, 128], fp16, tag="s16b")
            n_a = 3 * SLICES + LP1 // 128            # 240
            with tc.tile_pool(name="spsum", bufs=1, space="PSUM") as spsum:
                banks = [spsum.tile([128, 128], fp32, tag=f"s{b}",
                                    name=f"sbank{b}") for b in range(4)]
                idx = 0
                for t in range(N_CHUNK):
                    top = SLICES if t < 3 else LP1 // 128
                    for s in range(top):
                        sl = slice(s * 128, (s + 1) * 128)
                        nc.tensor.matmul(
                            banks[idx % 4][:],
                            lhsT=rad_t[t][:, sl],
                            rhs=att_t[t][:, sl],
                            start=(idx < 4),
                            stop=(idx >= n_a - 4),
                        )
                        idx += 1
                acc = small.tile([128, 128], fp32, tag="acc")
                nc.vector.tensor_copy(acc[:], banks[0][:])
                for b in range(1, 4):
                    nc.vector.tensor_add(acc[:], acc[:], banks[b][:])
                nc.vector.tensor_copy(s16a[:], acc[:])

            with (
                tc.tile_pool(name="wpsum", bufs=1, space="PSUM") as wpsum,
                tc.tile_pool(name="lpsum", bufs=1, space="PSUM") as lpsum,
            ):
                # a-part selection matmuls (off the critical path)
                w_ps = []
                for C in range(4):
                    w = wpsum.tile([64, 32], fp32, tag=f"w{C}", name=f"w{C}")
                    w_ps.append(w)
                    for g in range(4):
                        cs = (C * 4 + g) * 64
                        nc.tensor.matmul(w[:], lhsT=sel_sb[:, cs:cs + 64],
                                         rhs=s16a[:, g * 32:(g + 1) * 32],
                                         start=(g == 0), stop=False)

                # b-part: last 16 slices into 2 fresh banks
                lbs = [lpsum.tile([128, 128], fp32, tag=f"lb{b}",
                                  name=f"lb{b}") for b in range(2)]
                n_b = (CHUNK_COLS - LP1) // 128          # 16
                for j, s in enumerate(range(LP1 // 128, SLICES)):
                    sl = slice(s * 128, (s + 1) * 128)
                    nc.tensor.matmul(lbs[j % 2][:], lhsT=rad_t[3][:, sl],
                                     rhs=att_t[3][:, sl], start=(j < 2),
                                     stop=(j >= n_b - 2))
                accb = small.tile([128, 128], fp32, tag="accb")
                nc.vector.tensor_copy(accb[:], lbs[0][:])
                nc.vector.tensor_add(accb[:], accb[:], lbs[1][:])
                nc.vector.tensor_copy(s16b[:], accb[:])

                # b-part selection matmuls accumulate into the same W
                wh = small.tile([64, 128], fp16, tag="wh")
                for C in range(4):
                    for g in range(4):
                        cs = (C * 4 + g) * 64
                        nc.tensor.matmul(w_ps[C][:],
                                         lhsT=sel_sb[:, cs:cs + 64],
                                         rhs=s16b[:, g * 32:(g + 1) * 32],
                                         start=False, stop=(g == 3))
                    nc.vector.tensor_scalar_mul(
                        wh[:, C * 32:(C + 1) * 32], w_ps[C][:], float(SCALE))

            # ---- phase 3: csi chunks over F (8 x 1024 cols) ----
            csi_sb = small.tile([2, F], fp16, tag="csi")
            with (
                tc.tile_pool(name="tpsum", bufs=2, space="PSUM") as tpsum,
                tc.tile_pool(name="cpsum", bufs=2, space="PSUM") as cpsum,
            ):
                e_tiles = [None] * N_FCHUNK

                def issue_t(ci):
                    fs = slice(ci * FCHUNK, (ci + 1) * FCHUNK)
                    t_ps = tpsum.tile([128, FCHUNK], fp32, tag="t",
                                      name=f"t{ci}")
                    for k in (0, 1):
                        ks = slice(k * 512, (k + 1) * 512)
                        gs = slice(ci * FCHUNK + k * 512,
                                   ci * FCHUNK + (k + 1) * 512)
                        nc.tensor.matmul(t_ps[:, ks], lhsT=wh[:],
                                         rhs=gtd_sb[0:64, gs],
                                         start=True, stop=True)
                    e_sb = epool.tile([128, FCHUNK], fp16, tag=f"e{ci}",
                                      name=f"e{ci}")
                    nc.vector.tensor_mul(e_sb[:], gtd_sb[:, fs], t_ps[:])
                    e_tiles[ci] = e_sb

                def issue_ones(ci):
                    fs = slice(ci * FCHUNK, (ci + 1) * FCHUNK)
                    c_ps = cpsum.tile([2, FCHUNK], fp32, tag="c",
                                      name=f"c{ci}")
                    for k in (0, 1):
                        ks = slice(k * 512, (k + 1) * 512)
                        nc.tensor.matmul(c_ps[:, ks],
                                         lhsT=sel_sb[:, 1024:1026],
                                         rhs=e_tiles[ci][:, ks],
                                         start=True, stop=True)
                    nc.scalar.copy(csi_sb[:, fs], c_ps[:])
                    if ci % 2 == 1:
                        fs2 = slice((ci - 1) * FCHUNK, (ci + 1) * FCHUNK)
                        eng = nc.sync if (ci // 2) % 2 == 0 else nc.scalar
                        eng.dma_start(out_d[:, fs2], csi_sb[:, fs2])

                issue_t(0)
                for ci in range(1, N_FCHUNK):
                    issue_t(ci)
                    issue_ones(ci - 1)
                issue_ones(N_FCHUNK - 1)

    nc.compile()
    return nc


def _pack_core(arr, core):
    """Core's complex64 shard -> (N_CHUNK, 128, CHUNK_COLS) fp16 with
    per-row [Re(32) | Im(32)] packing."""
    sh = arr[core * DIR_PER_CORE:(core + 1) * DIR_PER_CORE].reshape(-1, R)
    a = sh.real.astype(np.float16)
    b = sh.imag.astype(np.float16)
    rows = np.concatenate([a, b], axis=1)            # (65536, 64)
    return np.ascontiguousarray(rows.reshape(N_CHUNK, 128, CHUNK_COLS))


def _prep_gtd(fbv):
    """(128, F) f16: [Re.T(32); Im.T(32)] duplicated to 128 partitions."""
    fb = np.asarray(fbv)
    gbt = np.concatenate([fb.real.T, fb.imag.T], axis=0).astype(np.float16)
    return np.ascontiguousarray(np.concatenate([gbt, gbt], axis=0))


def _build_in_maps(attenuation_vectors, radiation_vectors,
                   frequency_basis_vectors):
    gtd = _prep_gtd(frequency_basis_vectors)
    sel = _build_sel()
    in_maps = []
    for c in range(N_CORES):
        in_maps.append({
            "rad": _pack_core(radiation_vectors, c),
            "att": _pack_core(attenuation_vectors, c),
            "gtd": gtd, "sel": sel,
        })
    return in_maps


def kernel(attenuation_vectors, radiation_vectors, frequency_basis_vectors):
    from concourse.bass_utils import run_bass_kernel_spmd

    if "nc" not in _NC_CACHE:
        _NC_CACHE["nc"] = build_nc()
    nc = _NC_CACHE["nc"]

    in_maps = _build_in_maps(attenuation_vectors, radiation_vectors,
                             frequency_basis_vectors)
    res = run_bass_kernel_spmd(nc, in_maps, core_ids=list(range(N_CORES)))
    acc = np.zeros((2, F), np.float64)
    for r in res.results:
        acc += r["csi"].astype(np.float64)
    return (acc[0] + 1j * acc[1]).astype(np.complex64)


# revision 7
# speedup vs baseline: 1.1553x; 1.1553x over previous
"""Trainium2 Bass kernel for LowRankRayTracer.

csi[f] = (delta_t/D) * v_f^T M v_f,  M = conj(rad)^T conj(att)  (R=32, complex)
contracted over N = D*K = 524288 rows.

Strategy (8 cores, memory-bound => minimize HBM bytes):
  - Tolerance is 2e-2; fp16-only inputs give ~6e-4 (validated in sim), so
    each f32 component ships as ONE fp16 (half the hi+lo baseline's
    traffic): 16 MiB/core of ray data + ~2.3 MiB of frequency/const data.
    All input DMAs are issued up front into statically-resident SBUF
    tiles (20 MiB), so the input stream runs at the ~358 GB/s HBM limit
    without stalls.
  - Rows sharded 8 ways; host sums the 8 partial csi vectors (an
    on-device all-reduce was measured at ~60us on this path - dead end).
  - s128 = rad_pack^T att_pack accumulates in 4 PSUM banks for the first
    15.75 MiB; that partial is folded and pushed through the 16 +-1
    selection matmuls into W's PSUM accumulators while the last 0.5 MiB
    still streams, so only a small fold+16 matmuls remain on the post-DMA
    critical path (W = SEL(s_a) + SEL(s_b) - SEL is linear).
  - Phase 3 per 1024-col chunk: T = W^T g (PE), e = g (.) T (DVE, direct
    from PSUM), csi = ones^T e (PE, issue-interleaved with T so the
    in-order PE queue pipelines), (2,1024) csi copies on the otherwise
    idle ACT engine, output DMA in 4 overlapping pieces.
"""

import numpy as np

D, K, R = 4096, 128, 32
F = 8192
N_CORES = 8
DIR_PER_CORE = D // N_CORES              # 512
N_CHUNK = 4                              # macro chunks per tensor per core
CHUNK_COLS = 8192                        # fp16 per partition per chunk
SLICES = CHUNK_COLS // 128               # 64 matmul slices per chunk
SCALE = (200.0 / K) / D                  # delta_t / num_directions
FCHUNK = 512
N_FCHUNK = F // FCHUNK                   # 16
SEL_COLS = 576                           # 8*64 Wr-selection cols + ones2 + pad
LP1, LP2 = 6144, 7680                    # last-chunk DMA/PSUM split points

_NC_CACHE = {}


def _build_sel():
    """(128, 1152) f16: 16 (128,64) +-1 selection matrices + ones2 cols.

    W 32-block (R,C) = sum of sigma*Q_uv;  Q_uv = S64[32u:+32, 32v:+32],
    S64 = s128[0:64,0:64] + s128[64:,64:].  Matmul g uses rhs cols
    [32g:32g+32] of s128; source (side, v) -> g = v + 2*side.
    """
    table = {
        (0, 0): [(+1, 0, 0), (-1, 1, 1)],   # Mr
        (1, 0): [(+1, 0, 1), (+1, 1, 0)],   # -Mi
        (0, 1): [(+1, 0, 1), (+1, 1, 0)],   # -Mi
        (1, 1): [(-1, 0, 0), (+1, 1, 1)],   # -Mr
    }
    sel = np.zeros((128, SEL_COLS), np.float16)
    for (Rb, C), terms in table.items():
        for sigma, u, v in terms:
            for side in (0, 1):
                g = v + 2 * side
                for r0 in range(32):
                    p = 64 * side + 32 * u + r0
                    sel[p, (C * 4 + g) * 64 + 32 * Rb + r0] = sigma
    sel[0:64, 512] = 1.0
    sel[64:128, 513] = 1.0
    return sel


def build_nc():
    import concourse.bacc as bacc
    import concourse.mybir as mybir
    import concourse.tile as tile

    fp32 = mybir.dt.float32
    fp16 = mybir.dt.float16
    nc = bacc.Bacc(trn_type="TRN2", target_bir_lowering=False, debug=False)

    rad_d = nc.dram_tensor("rad", [N_CHUNK, 128, CHUNK_COLS], fp16,
                           kind="ExternalInput").ap()
    att_d = nc.dram_tensor("att", [N_CHUNK, 128, CHUNK_COLS], fp16,
                           kind="ExternalInput").ap()
    gtd_d = nc.dram_tensor("gtd", [128, F], fp16, kind="ExternalInput").ap()
    sel_d = nc.dram_tensor("sel", [128, SEL_COLS], fp16,
                           kind="ExternalInput").ap()
    out_d = nc.dram_tensor("csi", [2, F], fp16, kind="ExternalOutput").ap()

    with tile.TileContext(nc) as tc:
        with (
            tc.tile_pool(name="io", bufs=1) as io_pool,
            tc.tile_pool(name="small", bufs=1) as small,
            tc.tile_pool(name="epool", bufs=1) as epool,
        ):
            sel_sb = small.tile([128, SEL_COLS], fp16, tag="sel")
            gtd_sb = small.tile([128, F], fp16, tag="gtd")

            rad_t = [io_pool.tile([128, CHUNK_COLS], fp16, tag=f"rad{t}",
                                  name=f"rad{t}")
                     for t in range(N_CHUNK)]
            att_t = [io_pool.tile([128, CHUNK_COLS], fp16, tag=f"att{t}",
                                  name=f"att{t}")
                     for t in range(N_CHUNK)]

            # ---- all input DMAs up front; rad on the sync HWDGE ring,
            # att on the scalar ring (rings drain round-robin, sharing the
            # 358 GB/s HBM limit). sel/gtd ride mid-stream so the epilogue
            # and phase 3 never wait on them; the last chunk is split so
            # only 0.125 MiB of matmul input lands after the stream ends.
            hm = CHUNK_COLS // 2
            nc.sync.dma_start(rad_t[0][:, 0:hm], rad_d[0, :, 0:hm])
            nc.scalar.dma_start(att_t[0][:, 0:hm], att_d[0, :, 0:hm])
            nc.sync.dma_start(rad_t[0][:, hm:], rad_d[0, :, hm:])
            nc.scalar.dma_start(att_t[0][:, hm:], att_d[0, :, hm:])
            nc.sync.dma_start(sel_sb[:], sel_d[:])
            nc.sync.dma_start(rad_t[1][:], rad_d[1, :, :])
            nc.scalar.dma_start(att_t[1][:], att_d[1, :, :])
            nc.sync.dma_start(gtd_sb[:, 0:4096], gtd_d[:, 0:4096])
            nc.scalar.dma_start(gtd_sb[:, 4096:8192], gtd_d[:, 4096:8192])
            nc.sync.dma_start(rad_t[2][:], rad_d[2, :, :])
            nc.scalar.dma_start(att_t[2][:], att_d[2, :, :])
            nc.sync.dma_start(rad_t[3][:, 0:LP1], rad_d[3, :, 0:LP1])
            nc.scalar.dma_start(att_t[3][:, 0:LP1], att_d[3, :, 0:LP1])
            nc.sync.dma_start(rad_t[3][:, LP1:LP2], rad_d[3, :, LP1:LP2])
            nc.scalar.dma_start(att_t[3][:, LP1:LP2], att_d[3, :, LP1:LP2])
            nc.sync.dma_start(rad_t[3][:, LP2:], rad_d[3, :, LP2:])
            nc.scalar.dma_start(att_t[3][:, LP2:], att_d[3, :, LP2:])

            # ---- main accumulation: s128 += rad_slice^T att_slice ----
            # chunks 0-2 + chunk3 cols [0:LP1] -> banks 0-3 ("a" part);
            # chunk3 cols [LP1:] -> 2 fresh tiles ("b" part) so the a-fold
            # and 16 a-selection matmuls overlap the tail DMAs.
            s16a = small.tile([128, 128], fp16, tag="s16a")
            s16b = small.tile([128, 128], fp16, tag="s16b")
            n_a = 3 * SLICES + LP1 // 128            # 240
            with tc.tile_pool(name="spsum", bufs=1, space="PSUM") as spsum:
                banks = [spsum.tile([128, 128], fp32, tag=f"s{b}",
                                    name=f"sbank{b}") for b in range(4)]
                idx = 0
                for t in range(N_CHUNK):
                    top = SLICES if t < 3 else LP1 // 128
                    for s in range(top):
                        sl = slice(s * 128, (s + 1) * 128)
                        nc.tensor.matmul(
                            banks[idx % 4][:],
                            lhsT=rad_t[t][:, sl],
                            rhs=att_t[t][:, sl],
                            start=(idx < 4),
                            stop=(idx >= n_a - 4),
                        )
                        idx += 1
                acc = small.tile([128, 128], fp32, tag="acc")
                nc.vector.tensor_copy(acc[:], banks[0][:])
                for b in range(1, 4):
                    nc.vector.tensor_add(acc[:], acc[:], banks[b][:])
                nc.vector.tensor_copy(s16a[:], acc[:])

            with (
                tc.tile_pool(name="wpsum", bufs=1, space="PSUM") as wpsum,
                tc.tile_pool(name="lpsum", bufs=1, space="PSUM") as lpsum,
            ):
                # a-part selection matmuls (off the critical path); all 16
                # SEL matmuls form ONE accumulation group on one bank - the
                # per-element has_written bits make region-interleaved
                # accumulation exact.
                w_all = wpsum.tile([64, 64], fp32, tag="w_all")
                for C in range(2):
                    for g in range(4):
                        cs = (C * 4 + g) * 64
                        nc.tensor.matmul(w_all[:, C * 32:(C + 1) * 32],
                                         lhsT=sel_sb[:, cs:cs + 64],
                                         rhs=s16a[:, g * 32:(g + 1) * 32],
                                         start=(C == 0 and g == 0),
                                         stop=False)

                # b-part: last 16 slices accumulate into one fresh bank
                lb0 = lpsum.tile([128, 128], fp32, tag="lb0", name="lb0")
                n_b = (CHUNK_COLS - LP1) // 128          # 16
                for j, s in enumerate(range(LP1 // 128, SLICES)):
                    sl = slice(s * 128, (s + 1) * 128)
                    nc.tensor.matmul(lb0[:], lhsT=rad_t[3][:, sl],
                                     rhs=att_t[3][:, sl], start=(j == 0),
                                     stop=(j == n_b - 1))
                nc.vector.tensor_copy(s16b[:], lb0[:])

                # b-part selection matmuls accumulate into the same W
                wh = small.tile([64, 128], fp16, tag="wh")
                for C in range(2):
                    for g in range(4):
                        cs = (C * 4 + g) * 64
                        nc.tensor.matmul(w_all[:, C * 32:(C + 1) * 32],
                                         lhsT=sel_sb[:, cs:cs + 64],
                                         rhs=s16b[:, g * 32:(g + 1) * 32],
                                         start=False,
                                         stop=(C == 1 and g == 3))
                nc.vector.tensor_scalar_mul(wh[:, 0:64], w_all[:],
                                            float(SCALE))
                nc.vector.tensor_copy(wh[:, 64:128], wh[:, 0:64])

            # ---- phase 3: csi chunks over F (8 x 1024 cols) ----
            csi_sb = small.tile([2, F], fp16, tag="csi")
            with (
                tc.tile_pool(name="tpsum", bufs=4, space="PSUM") as tpsum,
                tc.tile_pool(name="cpsum", bufs=4, space="PSUM") as cpsum,
            ):
                e_tiles = [None] * N_FCHUNK

                def issue_t(ci):
                    fs = slice(ci * FCHUNK, (ci + 1) * FCHUNK)
                    t_ps = tpsum.tile([128, FCHUNK], fp32, tag="t",
                                      name=f"t{ci}")
                    nc.tensor.matmul(t_ps[:], lhsT=wh[:],
                                     rhs=gtd_sb[0:64, fs],
                                     start=True, stop=True)
                    e_sb = epool.tile([128, FCHUNK], fp16, tag=f"e{ci}",
                                      name=f"e{ci}")
                    nc.vector.tensor_mul(e_sb[:], gtd_sb[:, fs], t_ps[:])
                    e_tiles[ci] = e_sb

                def issue_ones(ci):
                    fs = slice(ci * FCHUNK, (ci + 1) * FCHUNK)
                    c_ps = cpsum.tile([2, FCHUNK], fp32, tag="c",
                                      name=f"c{ci}")
                    nc.tensor.matmul(c_ps[:], lhsT=sel_sb[:, 512:514],
                                     rhs=e_tiles[ci][:],
                                     start=True, stop=True)
                    nc.scalar.copy(csi_sb[:, fs], c_ps[:])
                    if ci in (7, N_FCHUNK - 1):
                        fs2 = (slice(0, 4096) if ci == 7
                               else slice(4096, 8192))
                        nc.sync.dma_start(out_d[:, fs2], csi_sb[:, fs2])

                issue_t(0)
                for ci in range(1, N_FCHUNK):
                    issue_t(ci)
                    issue_ones(ci - 1)
                issue_ones(N_FCHUNK - 1)

    nc.compile()
    return nc


def _pack_core(arr, core):
    """Core's complex64 shard -> (N_CHUNK, 128, CHUNK_COLS) fp16 with
    per-row [Re(32) | Im(32)] packing."""
    sh = arr[core * DIR_PER_CORE:(core + 1) * DIR_PER_CORE].reshape(-1, R)
    a = sh.real.astype(np.float16)
    b = sh.imag.astype(np.float16)
    rows = np.concatenate([a, b], axis=1)            # (65536, 64)
    return np.ascontiguousarray(rows.reshape(N_CHUNK, 128, CHUNK_COLS))


def _prep_gtd(fbv):
    """(128, F) f16: rows 0:64 = [Re.T; Im.T] = g, rows 64:128 = K g =
    [Im.T; -Re.T]  (Wi = Wr K, so the imag part reuses Wr with K g)."""
    fb = np.asarray(fbv)
    gbt = np.concatenate([fb.real.T, fb.imag.T], axis=0).astype(np.float16)
    kg = np.concatenate([fb.imag.T, -fb.real.T], axis=0).astype(np.float16)
    return np.ascontiguousarray(np.concatenate([gbt, kg], axis=0))


def _build_in_maps(attenuation_vectors, radiation_vectors,
                   frequency_basis_vectors):
    gtd = _prep_gtd(frequency_basis_vectors)
    sel = _build_sel()
    in_maps = []
    for c in range(N_CORES):
        in_maps.append({
            "rad": _pack_core(radiation_vectors, c),
            "att": _pack_core(attenuation_vectors, c),
            "gtd": gtd, "sel": sel,
        })
    return in_maps


def kernel(attenuation_vectors, radiation_vectors, frequency_basis_vectors):
    from concourse.bass_utils import run_bass_kernel_spmd

    if "nc" not in _NC_CACHE:
        _NC_CACHE["nc"] = build_nc()
    nc = _NC_CACHE["nc"]

    in_maps = _build_in_maps(attenuation_vectors, radiation_vectors,
                             frequency_basis_vectors)
    res = run_bass_kernel_spmd(nc, in_maps, core_ids=list(range(N_CORES)))
    acc = np.zeros((2, F), np.float64)
    for r in res.results:
        acc += r["csi"].astype(np.float64)
    return (acc[0] + 1j * acc[1]).astype(np.complex64)
